# revision 14
# baseline (speedup 1.0000x reference)
"""Trainium2 Bass kernel for nn_AttentionLayer_13288628814002.

Reference semantics (verified bit-exact vs the jax oracle):
    q_res = q @ Wq.T                    # [B, N, dk]   (irrelevant, see below)
    scores = (q_res/sqrt(dk)) @ k       # [B, N, 1]
    attn_weights = softmax(scores, -1)  # softmax over a SIZE-1 axis -> exactly 1.0
    v_res = v @ Wv.T                    # [B, 1, dv]
    out = attn_weights @ v_res          # [B, N, dv] == v_res broadcast over N

So:
    attn_weights == ones([B, N, 1])                      (exact)
    out[b, n, d] == v[b,0,0] * Wv[d,0]                   (exact, single f32 multiply)

The problem is a pure memory-regime broadcast-write of a 258 MiB output.
Sharding: batch dim B=32 across 8 cores (4 batches/core), weights replicated.
Each core:
  - loads its v shard (4 floats) + Wv (128 floats)
  - computes v_res rows on-chip (PE broadcast matmul: ones[1,128].T @ (v[b]*Wv
    repeated)[1,512] -> PSUM[128,512]) and replicates with DVE doubling copies
    into [128, F] SBUF tiles whose flat layout equals the row-major out[b]
    stream (period-128 pattern)
  - streams the tiles out with large contiguous DMAs (HWDGE). The first
    batch uses ascending chunk sizes so the first DMA launches early.
  - memsets + DMAs the all-ones attn shard.
Measured steady-state: ~369 GB/s/core of HBM writes == the documented
per-NeuronCore HBM roofline (~358-368 GB/s).
"""

import numpy as np

import concourse.bacc as bacc
import concourse.tile as tile
import concourse.mybir as mybir
from concourse.bass_utils import run_bass_kernel_spmd

B, N, DK, DV = 32, 16384, 128, 128
NCORES = 8
BPC = B // NCORES            # batches per core = 4

P = 128                      # SBUF partitions
TOT = N * DV                 # f32 elems per batch = 2097152 (= P * 16384)
ATTN_F = (BPC * N) // P      # 512

_f32 = mybir.dt.float32

# mode "copy": per-batch [P, maxchunk] tile built by DVE doubling copies.
# mode "bcast": per-batch [P, baseF] tile (one tensor_scalar from a prolog
#   pattern tile); out-DMAs use a stride-0 source AP so the DMA engines
#   replicate the base tile into the full chunk.
# steady/first: per-partition f32 elems per out-DMA; each sums to TOT//P.
VARIANTS = {
    "default": dict(steady=(8192, 8192), first=(512, 512, 1024, 2048, 4096, 8192), mode="copy"),
    "flat8k": dict(steady=(8192, 8192), first=(8192, 8192), mode="copy"),
    "a4k": dict(steady=(4096,) * 4, first=(512, 512, 1024, 2048, 4096, 8192), mode="copy"),
    "flat4k": dict(steady=(4096,) * 4, first=(4096,) * 4, mode="copy"),
    "bcast512": dict(steady=(8192, 8192), first=(8192, 8192), mode="bcast", baseF=512),
    "bcast2048": dict(steady=(8192, 8192), first=(8192, 8192), mode="bcast", baseF=2048),
    "bcast4k512": dict(steady=(4096,) * 4, first=(4096,) * 4, mode="bcast", baseF=512),
    # per-batch matmul (like copy mode) but stride-0 chunk DMAs from the
    # [P, 512] base — no doubling copies at all, shortest fill
    "bcastmm": dict(steady=(8192, 8192), first=(8192, 8192), mode="bcastmm"),
}


def _build_program(repeat=1, variant="default"):
    """repeat>1 re-emits the output-writing work `repeat` times (idempotent
    rewrites of the same destinations) — used by the test harness to estimate
    per-iteration HW time from wall-clock deltas."""
    cfg = VARIANTS[variant]
    steady, first, mode = cfg["steady"], cfg["first"], cfg["mode"]
    baseF = cfg.get("baseF", 512)
    assert sum(steady) == TOT // P and sum(first) == TOT // P

    nc = bacc.Bacc(
        "TRN2",
        target_bir_lowering=False,
        debug=False,
        num_devices=NCORES,
    )

    v_ap = nc.dram_tensor("v_shard", [1, BPC], _f32, kind="ExternalInput").ap()
    wv_ap = nc.dram_tensor("wv", [1, DV], _f32, kind="ExternalInput").ap()
    # row-major [BPC, TOT] == the flat [BPC, N, DV] out shard
    out_ap = nc.dram_tensor("out_shard", [BPC, TOT], _f32, kind="ExternalOutput").ap()
    # [P, ATTN_F] row-major == flat [BPC, N, 1] attn shard (content is all-ones)
    attn_ap = nc.dram_tensor("attn_shard", [P, ATTN_F], _f32, kind="ExternalOutput").ap()

    with tile.TileContext(nc) as tc:
        with (
            tc.tile_pool(name="const", bufs=1) as const_pool,
            tc.tile_pool(name="rhs", bufs=2) as rhs_pool,
            tc.tile_pool(name="psum", bufs=2, space="PSUM") as psum_pool,
            tc.tile_pool(name="big", bufs=2) as big_pool,
        ):
            # attn output: all ones
            attn_ones = const_pool.tile([P, ATTN_F], _f32)
            nc.vector.memset(attn_ones[:, :], 1.0)
            nc.sync.dma_start(attn_ap[:, :], attn_ones[:, :])

            # tiny inputs (scalar = the other HWDGE ring, so both issue at once)
            v_sb = const_pool.tile([1, BPC], _f32)
            nc.scalar.dma_start(v_sb[:, :], v_ap[:, :])
            # Wv repeated 4x along the free dim on one partition -> [1, 512]
            wv_rep = const_pool.tile([1, 4 * DV], _f32)
            nc.sync.dma_start(wv_rep[:, 0:DV], wv_ap[:, :])
            nc.vector.tensor_copy(wv_rep[:, DV:2 * DV], wv_rep[:, 0:DV])
            nc.vector.tensor_copy(wv_rep[:, 2 * DV:4 * DV], wv_rep[:, 0:2 * DV])

            # lhsT of ones: K=1 partition, M=128 free -> matmul broadcasts a
            # [1, 512] row into all 128 PSUM partitions.
            ones_k = const_pool.tile([1, P], _f32)
            nc.vector.memset(ones_k[:, :], 1.0)

            if mode == "bcastmm":
                for rep in range(repeat):
                    for b in range(BPC):
                        chunks = first if (rep == 0 and b == 0) else steady
                        rhs = rhs_pool.tile([1, 4 * DV], _f32)
                        nc.vector.tensor_scalar_mul(rhs[:, :], wv_rep[:, :], v_sb[:, b:b + 1])
                        ps = psum_pool.tile([P, 4 * DV], _f32)
                        nc.tensor.matmul(ps[:, :], ones_k[:, :], rhs[:, :], start=True, stop=True)
                        base = big_pool.tile([P, 4 * DV], _f32)
                        nc.vector.tensor_copy(base[:, :], ps[:, :])
                        off = 0
                        for cF in chunks:
                            r = cF // (4 * DV)
                            dest = out_ap[b, off:off + P * cF].rearrange(
                                "(p r f) -> p r f", p=P, r=r)
                            src = base[:, :][:, None, :].broadcast_to((P, r, 4 * DV))
                            nc.sync.dma_start(dest, src)
                            off += P * cF
            elif mode == "bcast":
                # pattern[p, j] = Wv[j mod 128] on every partition, width baseF
                ps_p = psum_pool.tile([P, 4 * DV], _f32)
                nc.tensor.matmul(ps_p[:, :], ones_k[:, :], wv_rep[:, :], start=True, stop=True)
                pattern = const_pool.tile([P, baseF], _f32)
                nc.vector.tensor_copy(pattern[:, 0:4 * DV], ps_p[:, :])
                sz = 4 * DV
                while sz < baseF:
                    cp = min(sz, baseF - sz)
                    nc.vector.tensor_copy(pattern[:, sz:sz + cp], pattern[:, 0:cp])
                    sz += cp
                # v_bc[p, b] = v[b] on every partition
                ps_v = psum_pool.tile([P, BPC], _f32)
                nc.tensor.matmul(ps_v[:, :], ones_k[:, :], v_sb[:, :], start=True, stop=True)
                v_bc = const_pool.tile([P, BPC], _f32)
                nc.vector.tensor_copy(v_bc[:, :], ps_v[:, :])

                for rep in range(repeat):
                    for b in range(BPC):
                        chunks = first if (rep == 0 and b == 0) else steady
                        base = big_pool.tile([P, baseF], _f32)
                        nc.vector.tensor_scalar_mul(base[:, :], pattern[:, :], v_bc[:, b:b + 1])
                        off = 0
                        for cF in chunks:
                            r = cF // baseF
                            dest = out_ap[b, off:off + P * cF].rearrange(
                                "(p r f) -> p r f", p=P, r=r)
                            src = base[:, :][:, None, :].broadcast_to((P, r, baseF))
                            nc.sync.dma_start(dest, src)
                            off += P * cF
            else:
                for rep in range(repeat):
                    for b in range(BPC):
                        chunks = first if (rep == 0 and b == 0) else steady
                        # rhs[0, j] = v[b] * Wv[j mod 128], j in [0, 512)
                        rhs = rhs_pool.tile([1, 4 * DV], _f32)
                        nc.vector.tensor_scalar_mul(rhs[:, :], wv_rep[:, :], v_sb[:, b:b + 1])

                        ps = psum_pool.tile([P, 4 * DV], _f32)
                        nc.tensor.matmul(ps[:, :], ones_k[:, :], rhs[:, :], start=True, stop=True)

                        # big[p, j] = v[b] * Wv[j mod 128] for every partition p,
                        # grown by doubling copies; DMA each chunk as soon as the
                        # tile content covers it.
                        bigF = max(chunks)
                        big = big_pool.tile([P, bigF], _f32)
                        nc.vector.tensor_copy(big[:, 0:4 * DV], ps[:, :])
                        sz = 4 * DV
                        off = 0
                        for cF in chunks:
                            while sz < cF:
                                cp = min(sz, cF - sz)
                                nc.vector.tensor_copy(big[:, sz:sz + cp], big[:, 0:cp])
                                sz += cp
                            dest = out_ap[b, off:off + P * cF].rearrange("(p f) -> p f", p=P)
                            nc.sync.dma_start(dest, big[:, 0:cF])
                            off += P * cF

    nc.compile()
    return nc


_PROGRAMS = {}


def _get_program(repeat=1, variant="default"):
    key = (repeat, variant)
    if key not in _PROGRAMS:
        _PROGRAMS[key] = _build_program(repeat, variant)
    return _PROGRAMS[key]


def _run(inputs, trace=False, repeat=1, variant="default"):
    """Run the SPMD bass kernel. Returns ((out, attn), BassKernelResults)."""
    v = np.ascontiguousarray(np.asarray(inputs["v"], dtype=np.float32))
    wv = np.ascontiguousarray(np.asarray(inputs["Wv"], dtype=np.float32))

    nc = _get_program(repeat, variant)
    in_maps = [
        {
            "v_shard": v[i * BPC:(i + 1) * BPC].reshape(1, BPC),
            "wv": wv.reshape(1, DV),
        }
        for i in range(NCORES)
    ]
    res = run_bass_kernel_spmd(nc, in_maps, list(range(NCORES)), trace=trace)

    out = np.empty((B, N, DV), dtype=np.float32)
    attn = np.empty((B, N, 1), dtype=np.float32)
    for i in range(NCORES):
        out[i * BPC:(i + 1) * BPC] = res.results[i]["out_shard"].reshape(BPC, N, DV)
        attn[i * BPC:(i + 1) * BPC] = res.results[i]["attn_shard"].reshape(BPC, N, 1)
    return (out, attn), res


def kernel(**inputs):
    (out, attn), _ = _run(inputs, trace=False)
    return (out, attn)


# revision 18
# speedup vs baseline: 1.0257x; 1.0257x over previous
"""Trainium2 Bass kernel for nn_AttentionLayer_13288628814002.

Reference semantics (verified bit-exact vs the jax oracle):
    q_res = q @ Wq.T                    # [B, N, dk]   (irrelevant, see below)
    scores = (q_res/sqrt(dk)) @ k       # [B, N, 1]
    attn_weights = softmax(scores, -1)  # softmax over a SIZE-1 axis -> exactly 1.0
    v_res = v @ Wv.T                    # [B, 1, dv]
    out = attn_weights @ v_res          # [B, N, dv] == v_res broadcast over N

So:
    attn_weights == ones([B, N, 1])                      (exact)
    out[b, n, d] == v[b,0,0] * Wv[d,0]                   (exact, single f32 multiply)

The problem is a pure memory-regime broadcast-write of a 258 MiB output.
Sharding: batch dim B=32 across 8 cores (4 batches/core), weights replicated.

Shipped design (mode "dbcast"): each core
  - loads v shard + Wv with stride-0 DRAM-source DMAs that land v on all 128
    partitions and Wv replicated x4 as a ready [128, 512] pattern tile (no
    PE/PSUM partition-broadcast needed)
  - per batch: one DVE tensor_scalar_mul (base = pattern * v[b]), then 4 MiB
    out-DMAs whose stride-0 source APs replicate the [128, 512] base into the
    full chunk (the flat layout of out[b] is a period-128 pattern)
  - memsets + DMAs the all-ones attn shard.
Measured steady-state: 355-412 GB/s/core of HBM writes across sessions ==
the per-NeuronCore sustained write wall (cost model derate: 368 GB/s).
One-shot (cost model, trace-verified structure): ~100.1 us = 5.4 us fill
(input-load latency bound) + 93.2 us gap-free stream + 1.5 us drain.
Earlier copy-mode design (PE broadcast matmul + DVE doubling copies,
ascending first-batch chunks) is retained as variant "default"; both are
bit-exact and steady-state equal within measurement resolution.
"""

import numpy as np

import concourse.bacc as bacc
import concourse.tile as tile
import concourse.mybir as mybir
from concourse.bass_utils import run_bass_kernel_spmd

B, N, DK, DV = 32, 16384, 128, 128
NCORES = 8
BPC = B // NCORES            # batches per core = 4

P = 128                      # SBUF partitions
TOT = N * DV                 # f32 elems per batch = 2097152 (= P * 16384)
ATTN_F = (BPC * N) // P      # 512

_f32 = mybir.dt.float32

# mode "copy": per-batch [P, maxchunk] tile built by DVE doubling copies.
# mode "bcast": per-batch [P, baseF] tile (one tensor_scalar from a prolog
#   pattern tile); out-DMAs use a stride-0 source AP so the DMA engines
#   replicate the base tile into the full chunk.
# steady/first: per-partition f32 elems per out-DMA; each sums to TOT//P.
VARIANTS = {
    "default": dict(steady=(8192, 8192), first=(512, 512, 1024, 2048, 4096, 8192), mode="copy"),
    "flat8k": dict(steady=(8192, 8192), first=(8192, 8192), mode="copy"),
    "a4k": dict(steady=(4096,) * 4, first=(512, 512, 1024, 2048, 4096, 8192), mode="copy"),
    "flat4k": dict(steady=(4096,) * 4, first=(4096,) * 4, mode="copy"),
    "bcast512": dict(steady=(8192, 8192), first=(8192, 8192), mode="bcast", baseF=512),
    "bcast2048": dict(steady=(8192, 8192), first=(8192, 8192), mode="bcast", baseF=2048),
    "bcast4k512": dict(steady=(4096,) * 4, first=(4096,) * 4, mode="bcast", baseF=512),
    # per-batch matmul (like copy mode) but stride-0 chunk DMAs from the
    # [P, 512] base — no doubling copies at all, shortest fill
    "bcastmm": dict(steady=(8192, 8192), first=(8192, 8192), mode="bcastmm"),
    # broadcast-load mode: v and Wv are DMA'd with stride-0 DRAM-source APs
    # straight into all 128 partitions (no PE/PSUM/ones at all); per batch a
    # single tensor_scalar_mul + stride-0 chunk DMAs. Shortest possible fill.
    "dbcast": dict(steady=(8192, 8192), first=(8192, 8192), mode="dbcast"),
}


def _build_program(repeat=1, variant="default"):
    """repeat>1 re-emits the output-writing work `repeat` times (idempotent
    rewrites of the same destinations) — used by the test harness to estimate
    per-iteration HW time from wall-clock deltas."""
    cfg = VARIANTS[variant]
    steady, first, mode = cfg["steady"], cfg["first"], cfg["mode"]
    baseF = cfg.get("baseF", 512)
    assert sum(steady) == TOT // P and sum(first) == TOT // P

    nc = bacc.Bacc(
        "TRN2",
        target_bir_lowering=False,
        debug=False,
        num_devices=NCORES,
    )

    v_ap = nc.dram_tensor("v_shard", [1, BPC], _f32, kind="ExternalInput").ap()
    wv_ap = nc.dram_tensor("wv", [1, DV], _f32, kind="ExternalInput").ap()
    # row-major [BPC, TOT] == the flat [BPC, N, DV] out shard
    out_ap = nc.dram_tensor("out_shard", [BPC, TOT], _f32, kind="ExternalOutput").ap()
    # [P, ATTN_F] row-major == flat [BPC, N, 1] attn shard (content is all-ones)
    attn_ap = nc.dram_tensor("attn_shard", [P, ATTN_F], _f32, kind="ExternalOutput").ap()

    if mode == "dbcast":
        with tile.TileContext(nc) as tc:
            with (
                tc.tile_pool(name="const", bufs=1) as const_pool,
                tc.tile_pool(name="base", bufs=2) as base_pool,
            ):
                # broadcast loads: stride-0 DRAM-source APs put v on all 128
                # partitions and Wv (replicated x4) into a [P, 512] pattern
                v_bc = const_pool.tile([P, BPC], _f32)
                nc.scalar.dma_start(v_bc[:, :], v_ap[0:1, :].broadcast_to((P, BPC)))
                pattern = const_pool.tile([P, 4 * DV], _f32)
                wsrc = wv_ap[0:1, None, :].broadcast_to((P, 4, DV))
                nc.sync.dma_start(pattern[:, :].rearrange("p (r f) -> p r f", r=4), wsrc)
                attn_ones = const_pool.tile([P, ATTN_F], _f32)
                nc.vector.memset(attn_ones[:, :], 1.0)
                nc.sync.dma_start(attn_ap[:, :], attn_ones[:, :])
                for rep in range(repeat):
                    for b in range(BPC):
                        base = base_pool.tile([P, 4 * DV], _f32)
                        nc.vector.tensor_scalar_mul(base[:, :], pattern[:, :], v_bc[:, b:b + 1])
                        off = 0
                        for cF in steady:
                            r = cF // (4 * DV)
                            dest = out_ap[b, off:off + P * cF].rearrange(
                                "(p r f) -> p r f", p=P, r=r)
                            nc.sync.dma_start(
                                dest, base[:, :][:, None, :].broadcast_to((P, r, 4 * DV)))
                            off += P * cF
        nc.compile()
        return nc

    with tile.TileContext(nc) as tc:
        with (
            tc.tile_pool(name="const", bufs=1) as const_pool,
            tc.tile_pool(name="rhs", bufs=2) as rhs_pool,
            tc.tile_pool(name="psum", bufs=2, space="PSUM") as psum_pool,
            tc.tile_pool(name="big", bufs=2) as big_pool,
        ):
            # attn output: all ones
            attn_ones = const_pool.tile([P, ATTN_F], _f32)
            nc.vector.memset(attn_ones[:, :], 1.0)
            nc.sync.dma_start(attn_ap[:, :], attn_ones[:, :])

            # tiny inputs (scalar = the other HWDGE ring, so both issue at once)
            v_sb = const_pool.tile([1, BPC], _f32)
            nc.scalar.dma_start(v_sb[:, :], v_ap[:, :])
            # Wv repeated 4x along the free dim on one partition -> [1, 512]
            wv_rep = const_pool.tile([1, 4 * DV], _f32)
            nc.sync.dma_start(wv_rep[:, 0:DV], wv_ap[:, :])
            nc.vector.tensor_copy(wv_rep[:, DV:2 * DV], wv_rep[:, 0:DV])
            nc.vector.tensor_copy(wv_rep[:, 2 * DV:4 * DV], wv_rep[:, 0:2 * DV])

            # lhsT of ones: K=1 partition, M=128 free -> matmul broadcasts a
            # [1, 512] row into all 128 PSUM partitions.
            ones_k = const_pool.tile([1, P], _f32)
            nc.vector.memset(ones_k[:, :], 1.0)

            if mode == "bcastmm":
                for rep in range(repeat):
                    for b in range(BPC):
                        chunks = first if (rep == 0 and b == 0) else steady
                        rhs = rhs_pool.tile([1, 4 * DV], _f32)
                        nc.vector.tensor_scalar_mul(rhs[:, :], wv_rep[:, :], v_sb[:, b:b + 1])
                        ps = psum_pool.tile([P, 4 * DV], _f32)
                        nc.tensor.matmul(ps[:, :], ones_k[:, :], rhs[:, :], start=True, stop=True)
                        base = big_pool.tile([P, 4 * DV], _f32)
                        nc.vector.tensor_copy(base[:, :], ps[:, :])
                        off = 0
                        for cF in chunks:
                            r = cF // (4 * DV)
                            dest = out_ap[b, off:off + P * cF].rearrange(
                                "(p r f) -> p r f", p=P, r=r)
                            src = base[:, :][:, None, :].broadcast_to((P, r, 4 * DV))
                            nc.sync.dma_start(dest, src)
                            off += P * cF
            elif mode == "bcast":
                # pattern[p, j] = Wv[j mod 128] on every partition, width baseF
                ps_p = psum_pool.tile([P, 4 * DV], _f32)
                nc.tensor.matmul(ps_p[:, :], ones_k[:, :], wv_rep[:, :], start=True, stop=True)
                pattern = const_pool.tile([P, baseF], _f32)
                nc.vector.tensor_copy(pattern[:, 0:4 * DV], ps_p[:, :])
                sz = 4 * DV
                while sz < baseF:
                    cp = min(sz, baseF - sz)
                    nc.vector.tensor_copy(pattern[:, sz:sz + cp], pattern[:, 0:cp])
                    sz += cp
                # v_bc[p, b] = v[b] on every partition
                ps_v = psum_pool.tile([P, BPC], _f32)
                nc.tensor.matmul(ps_v[:, :], ones_k[:, :], v_sb[:, :], start=True, stop=True)
                v_bc = const_pool.tile([P, BPC], _f32)
                nc.vector.tensor_copy(v_bc[:, :], ps_v[:, :])

                for rep in range(repeat):
                    for b in range(BPC):
                        chunks = first if (rep == 0 and b == 0) else steady
                        base = big_pool.tile([P, baseF], _f32)
                        nc.vector.tensor_scalar_mul(base[:, :], pattern[:, :], v_bc[:, b:b + 1])
                        off = 0
                        for cF in chunks:
                            r = cF // baseF
                            dest = out_ap[b, off:off + P * cF].rearrange(
                                "(p r f) -> p r f", p=P, r=r)
                            src = base[:, :][:, None, :].broadcast_to((P, r, baseF))
                            nc.sync.dma_start(dest, src)
                            off += P * cF
            else:
                for rep in range(repeat):
                    for b in range(BPC):
                        chunks = first if (rep == 0 and b == 0) else steady
                        # rhs[0, j] = v[b] * Wv[j mod 128], j in [0, 512)
                        rhs = rhs_pool.tile([1, 4 * DV], _f32)
                        nc.vector.tensor_scalar_mul(rhs[:, :], wv_rep[:, :], v_sb[:, b:b + 1])

                        ps = psum_pool.tile([P, 4 * DV], _f32)
                        nc.tensor.matmul(ps[:, :], ones_k[:, :], rhs[:, :], start=True, stop=True)

                        # big[p, j] = v[b] * Wv[j mod 128] for every partition p,
                        # grown by doubling copies; DMA each chunk as soon as the
                        # tile content covers it.
                        bigF = max(chunks)
                        big = big_pool.tile([P, bigF], _f32)
                        nc.vector.tensor_copy(big[:, 0:4 * DV], ps[:, :])
                        sz = 4 * DV
                        off = 0
                        for cF in chunks:
                            while sz < cF:
                                cp = min(sz, cF - sz)
                                nc.vector.tensor_copy(big[:, sz:sz + cp], big[:, 0:cp])
                                sz += cp
                            dest = out_ap[b, off:off + P * cF].rearrange("(p f) -> p f", p=P)
                            nc.sync.dma_start(dest, big[:, 0:cF])
                            off += P * cF

    nc.compile()
    return nc


_PROGRAMS = {}


def _get_program(repeat=1, variant="dbcast"):
    key = (repeat, variant)
    if key not in _PROGRAMS:
        _PROGRAMS[key] = _build_program(repeat, variant)
    return _PROGRAMS[key]


def _run(inputs, trace=False, repeat=1, variant="dbcast"):
    """Run the SPMD bass kernel. Returns ((out, attn), BassKernelResults)."""
    v = np.ascontiguousarray(np.asarray(inputs["v"], dtype=np.float32))
    wv = np.ascontiguousarray(np.asarray(inputs["Wv"], dtype=np.float32))

    nc = _get_program(repeat, variant)
    in_maps = [
        {
            "v_shard": v[i * BPC:(i + 1) * BPC].reshape(1, BPC),
            "wv": wv.reshape(1, DV),
        }
        for i in range(NCORES)
    ]
    res = run_bass_kernel_spmd(nc, in_maps, list(range(NCORES)), trace=trace)

    out = np.empty((B, N, DV), dtype=np.float32)
    attn = np.empty((B, N, 1), dtype=np.float32)
    for i in range(NCORES):
        out[i * BPC:(i + 1) * BPC] = res.results[i]["out_shard"].reshape(BPC, N, DV)
        attn[i * BPC:(i + 1) * BPC] = res.results[i]["attn_shard"].reshape(BPC, N, 1)
    return (out, attn), res


def kernel(**inputs):
    (out, attn), _ = _run(inputs, trace=False)
    return (out, attn)
